# revision 24
# baseline (speedup 1.0000x reference)
"""Trainium2 Bass kernel for the CP-sparse-degree-LU module.

Reference computation (all fp32):
    zf  = z.reshape(-1, 2048)                      # [N=8192, d]
    W   = masks * U                                # [6, k, d]
    out = zf @ W[0].T                              # [N, k]
    for i in 1..5: out = (zf @ W[i].T) * out + out
    x   = out @ C_w.T + C_b                        # [N, o]

Sharding: data-parallel over the token dim N across 8 cores (1024 tokens
each), weights replicated; no collectives. Everything is laid out
transposed on device (acc is [k, tok], output is [o, tok]) so the degree
chain and the final projection both run without on-device transposes:
    acc.T = W_i @ z.T  -> lhsT = W_i.T tiles [d,k], rhs = z.T [d, tok]
    x.T   = C_w @ acc  -> lhsT = C_w.T tiles [k,o], rhs = acc [k, tok]

Sparsity: W = masks*U is block-sparse (tril/triu factors plus a degree
mask that zeroes rank rows < i*K/DEGREE at degree i). The host detects
all-zero 128x128 blocks of the actual W at runtime and builds the device
program skipping them: a skipped (degree, rank-tile) group contributes
mm = 0, so acc = (0+1)*acc is the identity and the whole group (DMA,
matmuls, DVE update) is dropped. This is sound for arbitrary inputs —
only provably-zero blocks are skipped; dense inputs yield the dense
program.

Precision: matmul operands (z, W, C_w) are bf16 (host-side cast), PSUM
accumulation fp32, chain state acc bf16, final output fp32. Measured
end-to-end max error ~5e-3 of output scale vs the 2e-2 gate. bf16 runs
at the same PE rate as fp32r (1 col/cycle) but halves DMA traffic and
enables fast-weight-load so LDWEIGHTS fully hides under the matmuls.

Schedule details:
- A short warmup burst of dummy matmuls on zeroed SBUF trips the PE HAM
  clock-gate to full rate while the first input DMAs are in flight.
- The active weight blocks of each degree are packed contiguously in
  DRAM and loaded with a handful of large DMAs per degree (deep
  prefetch, few semaphores); z loads are split so the first real matmul
  only waits for 128 KB.
- z and acc live in one big SBUF tile each (subtile dependency tracking
  keeps scheduling fine-grained) to minimize tile-pool slots — the
  epilogue teardown ladder scales with slot count.
- PSUM->SBUF copies (degree 0) and the final bias-adds run on the
  Activation engine, keeping DVE off the critical path.
"""

import os
import sys
import types
from contextlib import ExitStack

import numpy as np

DEGREE, D, K, O = 6, 2048, 2048, 2048
N_CORES = 8
N_TOTAL = 8192
TOK = N_TOTAL // N_CORES  # 1024 tokens per core
P = 128
DT = D // P  # 16 contraction tiles (degree matmuls)
KT = K // P  # 16 rank tiles
OT = O // P  # 16 output tiles
NC_CHUNK = 512  # moving free dim per matmul (PSUM bank, fp32 max)
TC = TOK // NC_CHUNK  # 2 token chunks
N_WARMUP = 20  # dummy matmuls (N=256) to trip the PE clock-gate

_CACHE = {}


def _install_ntff_shim():
    """Register antenv.axon_hooks so run_bass_kernel_spmd(trace=True) can
    profile under axon. Safe no-op if anything is unavailable."""
    try:
        if "antenv.axon_hooks" in sys.modules:
            return
        mod = types.ModuleType("antenv.axon_hooks")
        mod._hook = None
        mod.set_axon_ntff_profile_hook = lambda h: setattr(mod, "_hook", h)
        mod.get_axon_ntff_profile_hook = lambda: mod._hook
        sys.modules["antenv.axon_hooks"] = mod
        from trn_agent_boot.trn_boot import _ntff_profile_via_ctypes

        mod._hook = _ntff_profile_via_ctypes("/opt/axon/libaxon_pjrt.so")
    except Exception:
        pass


def _degree_plan(ranges):
    """Per-degree packing plan from the block-sparsity ranges.

    Returns (plans, nblk_total) where plans[i] = (groups, nblk, chunks):
      groups: list of (kt, lo, hi, goff) — goff is the block offset of the
              group inside the degree's packed weight tile
      nblk:   total packed blocks for this degree
      chunks: list of (block_start, block_count) DMA splits (group-aligned)
    """
    plans = []
    total = 0
    for i in range(DEGREE):
        groups = []
        off = 0
        for kt in range(KT):
            rng = ranges[i][kt]
            if rng is None:
                continue
            lo, hi = rng
            groups.append((kt, lo, hi, off))
            off += hi - lo + 1
        nblk = off
        # DMA splits: degrees 0-1 are consumed while z is still streaming
        # in — per-group chunks keep the matmul stream fed at fine grain.
        # Later degrees prefetch far ahead of use: two chunks suffice.
        bounds = [g[3] for g in groups] + [nblk]
        if i <= 1:
            cuts = bounds[:-1]
        else:
            half = min(
                (b for b in bounds[1:-1] if b > 0),
                key=lambda b: abs(b - nblk // 2),
                default=None,
            )
            cuts = [0] + ([half] if half else [])
        cuts = sorted(set(c for c in cuts if c < nblk))
        chunks = [
            (c, (cuts[j + 1] if j + 1 < len(cuts) else nblk) - c)
            for j, c in enumerate(cuts)
        ]
        plans.append((groups, nblk, chunks))
        total += nblk
    return plans, total


def _build(ranges):
    """ranges[i][kt] = (dt_lo, dt_hi) inclusive active range, or None if the
    whole (degree, rank-tile) block row is zero."""
    import concourse.tile as tile
    from concourse import bacc, mybir

    f32 = mybir.dt.float32
    bf16 = mybir.dt.bfloat16
    ADD = mybir.AluOpType.add
    MULT = mybir.AluOpType.mult
    COPY = mybir.ActivationFunctionType.Copy
    IDENT = mybir.ActivationFunctionType.Identity

    plans, nblk_total = _degree_plan(ranges)

    nc = bacc.Bacc("TRN2", target_bir_lowering=False, debug=False)

    # z.T per core, tile-major so each dt slice is a contiguous 256 KB
    # DMA: [dt, di, t] = z[t, dt*P + di]
    z_d = nc.dram_tensor("z", [DT, P, TOK], bf16, kind="ExternalInput")
    # Packed active weight blocks, all degrees concatenated:
    # [di, (blk)*P + ki]; blk enumerates (degree, kt, dt) in program order.
    w_d = nc.dram_tensor("w", [P, nblk_total * P], bf16, kind="ExternalInput")
    # C_w tiled: [ot, ki, kt*P + oi] = C_w[ot*P+oi, kt*P+ki]
    c_d = nc.dram_tensor("c", [OT, P, KT * P], bf16, kind="ExternalInput")
    # C_b tiled: [oi, ot] = C_b[ot*P + oi]
    cb_d = nc.dram_tensor("cb", [P, OT], f32, kind="ExternalInput")
    # x.T tile-major: [ot, oi, t] = x.T[ot*P + oi, t]
    x_d = nc.dram_tensor("x", [OT, P, TOK], f32, kind="ExternalOutput")

    z_ap, w_ap, c_ap, cb_ap, x_ap = (t.ap() for t in (z_d, w_d, c_d, cb_d, x_d))

    with tile.TileContext(nc) as tc, ExitStack() as ctx:
        zpool = ctx.enter_context(tc.tile_pool(name="z", bufs=1))
        accpool = ctx.enter_context(tc.tile_pool(name="acc", bufs=1))
        wpool = ctx.enter_context(tc.tile_pool(name="w", bufs=2))
        cpool = ctx.enter_context(tc.tile_pool(name="c", bufs=4))
        cbpool = ctx.enter_context(tc.tile_pool(name="cb", bufs=1))
        xpool = ctx.enter_context(tc.tile_pool(name="xt", bufs=2))
        wupool = ctx.enter_context(tc.tile_pool(name="wu", bufs=1))
        pspool = ctx.enter_context(tc.tile_pool(name="ps", bufs=4, space="PSUM"))

        # Single resident tiles; subtile dep tracking keeps readers of each
        # region independent.
        z_sb = zpool.tile([P, DT * TOK], bf16, tag="z")
        acc_sb = accpool.tile([P, KT * TOK], bf16, tag="acc")
        cb_sb = cbpool.tile([P, OT], f32)

        def z_r(dt_, tcx_=None):
            if tcx_ is None:
                return z_sb[:, dt_ * TOK : (dt_ + 1) * TOK]
            lo_ = dt_ * TOK + tcx_ * NC_CHUNK
            return z_sb[:, lo_ : lo_ + NC_CHUNK]

        def acc_r(kt_, tcx_=None):
            if tcx_ is None:
                return acc_sb[:, kt_ * TOK : (kt_ + 1) * TOK]
            lo_ = kt_ * TOK + tcx_ * NC_CHUNK
            return acc_sb[:, lo_ : lo_ + NC_CHUNK]

        # PE warmup: a burst of dummy matmuls on zeroed SBUF keeps the PE
        # busy while the first input DMAs land, tripping the HAM clock-gate
        # to full rate before real work starts.
        wu = wupool.tile([P, 256], bf16)
        nc.gpsimd.memset(wu[:], 0.0)
        wups = pspool.tile([P, TOK], f32, tag="ps")
        for _ in range(N_WARMUP):
            nc.tensor.matmul(wups[:, :256], wu[:, :P], wu[:], start=True, stop=True)

        # Issue z DMAs lazily, right before the first group that reads each
        # dt slice; the early (head-critical) slices split per token chunk.
        # z is spread round-robin across otherwise-idle DMA queues — one
        # queue's share of HBM cannot keep up with degree-0 consumption.
        # z0 rides the sync queue ahead of every weight chunk so the very
        # first matmul group is fed as early as possible.
        nc.sync.dma_start(z_sb[:, 0:NC_CHUNK], z_ap[0][:, 0:NC_CHUNK])
        nc.sync.dma_start(z_sb[:, NC_CHUNK:TOK], z_ap[0][:, NC_CHUNK:TOK])
        z_issued = [False] * DT
        z_issued[0] = True
        z_queues = [nc.gpsimd, nc.scalar]

        def ensure_z(lo_, hi_):
            for dt_ in range(lo_, hi_ + 1):
                if not z_issued[dt_]:
                    q = z_queues[dt_ % len(z_queues)]
                    nsp = 2 if dt_ < 8 else 1
                    for cx in range(nsp):
                        w_ = TOK // nsp
                        q.dma_start(
                            z_sb[:, dt_ * TOK + cx * w_ : dt_ * TOK + (cx + 1) * w_],
                            z_ap[dt_][:, cx * w_ : (cx + 1) * w_],
                        )
                    z_issued[dt_] = True

        # Degree chain over acc[kt-block, tokens].
        wbase = 0
        for i in range(DEGREE):
            groups, nblk, chunks = plans[i]
            if nblk == 0:
                for kt in range(KT):
                    if ranges[i][kt] is None and i == 0:
                        nc.gpsimd.memset(acc_r(kt), 0.0)
                continue
            w_sb = wpool.tile([P, nblk * P], bf16, tag="w", name=f"wdeg{i}")
            for cs, cn in chunks:
                nc.sync.dma_start(
                    w_sb[:, cs * P : (cs + cn) * P],
                    w_ap[:, (wbase + cs) * P : (wbase + cs + cn) * P],
                )
            if i == 0:
                for kt in range(KT):
                    if ranges[i][kt] is None:
                        nc.gpsimd.memset(acc_r(kt), 0.0)
            for kt, lo, hi, goff in groups:
                ndt = hi - lo + 1
                ensure_z(lo, hi)
                ps = pspool.tile([P, TOK], f32, tag="ps")
                # Steady state: dt outer / chunk inner — each weight tile is
                # loaded once and feeds both token chunks back-to-back.
                # Early degree-0 groups run chunk-outer instead: their first
                # half only needs the (faster-arriving) first token chunk of
                # each z slice, easing the startup DMA crunch.
                early = i == 0 and kt < 8
                if early:
                    order = [(j, tcx) for tcx in range(TC) for j in range(ndt)]
                else:
                    order = [(j, tcx) for j in range(ndt) for tcx in range(TC)]
                for j, tcx in order:
                    nc.tensor.matmul(
                        ps[:, tcx * NC_CHUNK : (tcx + 1) * NC_CHUNK],
                        w_sb[:, (goff + j) * P : (goff + j + 1) * P],
                        z_r(lo + j, tcx),
                        start=(j == 0),
                        stop=(j == ndt - 1),
                    )
                if i == 0:
                    # PSUM -> SBUF copy on the Activation engine
                    nc.scalar.activation(acc_r(kt), ps[:], COPY)
                else:
                    # acc = (mm + 1) * acc  — one DVE op
                    nc.vector.scalar_tensor_tensor(
                        acc_r(kt), ps[:], 1.0, acc_r(kt), ADD, MULT
                    )
            wbase += nblk

        # Final projection: x.T[ot-block] = C_w @ acc + C_b
        nc.sync.dma_start(cb_sb[:], cb_ap)
        for ot in range(OT):
            c_sb = cpool.tile([P, KT * P], bf16, tag="c")
            nc.sync.dma_start(c_sb[:], c_ap[ot])
            ps = pspool.tile([P, TOK], f32, tag="ps")
            for kt in range(KT):
                for tcx in range(TC):
                    nc.tensor.matmul(
                        ps[:, tcx * NC_CHUNK : (tcx + 1) * NC_CHUNK],
                        c_sb[:, kt * P : (kt + 1) * P],
                        acc_r(kt, tcx),
                        start=(kt == 0),
                        stop=(kt == KT - 1),
                    )
            xt = xpool.tile([P, TOK], f32, tag="xt")
            # Finer-grained drain near the end so the final bias-add + DMA
            # pipeline empties quickly after the last matmul.
            nsplit = 4 if ot == OT - 1 else (2 if ot == OT - 2 else 1)
            step = TOK // nsplit
            for h in range(nsplit):
                sl = slice(h * step, (h + 1) * step)
                nc.scalar.activation(
                    xt[:, sl], ps[:, sl], IDENT, bias=cb_sb[:, ot : ot + 1]
                )
                # Spread x writes over three queues mid-phase; near the end
                # keep them off the scalar queue so the descriptor pushes
                # don't serialize with the final Activation ops.
                if ot < OT - 2:
                    xq = [nc.gpsimd, nc.sync, nc.scalar][ot % 3]
                else:
                    xq = nc.gpsimd if (ot + h) % 2 == 0 else nc.sync
                xq.dma_start(x_ap[ot][:, sl], xt[:, sl])

    nc.compile()
    return nc


def kernel(z, U, masks, C_w, C_b):
    import ml_dtypes
    from concourse.bass_utils import run_bass_kernel_spmd

    if os.environ.get("BASS_TRACE"):
        _install_ntff_shim()

    bf16 = ml_dtypes.bfloat16
    lead = z.shape[:-1]
    zf = np.ascontiguousarray(np.asarray(z, dtype=np.float32).reshape(-1, D))
    W = np.asarray(masks, dtype=np.float32) * np.asarray(U, dtype=np.float32)
    C_w = np.asarray(C_w, dtype=np.float32)
    C_b = np.asarray(C_b, dtype=np.float32)

    # Detect all-zero 128x128 blocks of W; build per-(degree, rank-tile)
    # contraction ranges. Only provably-zero blocks are skipped.
    blk = (
        np.abs(W.reshape(DEGREE, KT, P, DT, P)).max(axis=(2, 4)) > 0.0
    )  # [i, kt, dt]
    ranges = []
    for i in range(DEGREE):
        row = []
        for kt in range(KT):
            nz = np.flatnonzero(blk[i, kt])
            row.append((int(nz[0]), int(nz[-1])) if len(nz) else None)
        ranges.append(tuple(row))
    ranges = tuple(ranges)

    plans, nblk_total = _degree_plan(ranges)

    # Packed weight layout: for each degree, active groups' blocks
    # concatenated; block (i,kt,dt) -> [di, blk*P + ki].
    Wr = W.reshape(DEGREE, KT, P, DT, P)  # [i, kt, ki, dt, di]
    parts = []
    for i in range(DEGREE):
        for kt, lo, hi, _ in plans[i][0]:
            blkw = Wr[i, kt][:, lo : hi + 1, :]  # [ki, ndt, di]
            parts.append(blkw.transpose(2, 1, 0).reshape(P, -1))  # [di, ndt*P]
    w_dev = (
        np.concatenate(parts, axis=1).astype(bf16)
        if parts
        else np.zeros((P, 0), bf16)
    )
    w_dev = np.ascontiguousarray(w_dev)
    assert w_dev.shape[1] == nblk_total * P

    c_dev = np.ascontiguousarray(
        C_w.reshape(OT, P, KT, P).transpose(0, 3, 2, 1)
    ).reshape(OT, P, KT * P).astype(bf16)
    cb_dev = np.ascontiguousarray(C_b.reshape(OT, P).T)

    in_maps = []
    for c in range(N_CORES):
        zs = zf[c * TOK : (c + 1) * TOK]  # [TOK, D]
        z_dev = np.ascontiguousarray(zs.T.reshape(DT, P, TOK)).astype(bf16)
        in_maps.append({"z": z_dev, "w": w_dev, "c": c_dev, "cb": cb_dev})

    if _CACHE.get("ranges") != ranges:
        _CACHE["nc"] = _build(ranges)
        _CACHE["ranges"] = ranges
    nc = _CACHE["nc"]

    res = run_bass_kernel_spmd(nc, in_maps, core_ids=list(range(N_CORES)))
    _CACHE["last_result"] = res

    # per-core x is [OT, P, TOK]; rearrange to [TOK, O]
    parts = [
        res.results[c]["x"].reshape(O, TOK).T for c in range(N_CORES)
    ]
    x = np.concatenate(parts, axis=0)
    return x.reshape(*lead, O)


# revision 25
# speedup vs baseline: 1.0104x; 1.0104x over previous
"""Trainium2 Bass kernel for the CP-sparse-degree-LU module.

Reference computation (all fp32):
    zf  = z.reshape(-1, 2048)                      # [N=8192, d]
    W   = masks * U                                # [6, k, d]
    out = zf @ W[0].T                              # [N, k]
    for i in 1..5: out = (zf @ W[i].T) * out + out
    x   = out @ C_w.T + C_b                        # [N, o]

Sharding: data-parallel over the token dim N across 8 cores (1024 tokens
each), weights replicated; no collectives. Everything is laid out
transposed on device (acc is [k, tok], output is [o, tok]) so the degree
chain and the final projection both run without on-device transposes:
    acc.T = W_i @ z.T  -> lhsT = W_i.T tiles [d,k], rhs = z.T [d, tok]
    x.T   = C_w @ acc  -> lhsT = C_w.T tiles [k,o], rhs = acc [k, tok]

Sparsity: W = masks*U is block-sparse (tril/triu factors plus a degree
mask that zeroes rank rows < i*K/DEGREE at degree i). The host detects
all-zero 128x128 blocks of the actual W at runtime and builds the device
program skipping them: a skipped (degree, rank-tile) group contributes
mm = 0, so acc = (0+1)*acc is the identity and the whole group (DMA,
matmuls, DVE update) is dropped. This is sound for arbitrary inputs —
only provably-zero blocks are skipped; dense inputs yield the dense
program.

Precision: matmul operands (z, W, C_w) are bf16 (host-side cast), PSUM
accumulation fp32, chain state acc bf16, final output fp32. Measured
end-to-end max error ~5e-3 of output scale vs the 2e-2 gate. bf16 runs
at the same PE rate as fp32r (1 col/cycle) but halves DMA traffic and
enables fast-weight-load so LDWEIGHTS fully hides under the matmuls.

Schedule details:
- A short warmup burst of dummy matmuls on zeroed SBUF trips the PE HAM
  clock-gate to full rate while the first input DMAs are in flight.
- The active weight blocks of each degree are packed contiguously in
  DRAM and loaded with a handful of large DMAs per degree (deep
  prefetch, few semaphores); z loads are split so the first real matmul
  only waits for 128 KB.
- z and acc live in one big SBUF tile each (subtile dependency tracking
  keeps scheduling fine-grained) to minimize tile-pool slots — the
  epilogue teardown ladder scales with slot count.
- PSUM->SBUF copies (degree 0) and the final bias-adds run on the
  Activation engine, keeping DVE off the critical path.
"""

import os
import sys
import types
from contextlib import ExitStack

import numpy as np

DEGREE, D, K, O = 6, 2048, 2048, 2048
N_CORES = 8
N_TOTAL = 8192
TOK = N_TOTAL // N_CORES  # 1024 tokens per core
P = 128
DT = D // P  # 16 contraction tiles (degree matmuls)
KT = K // P  # 16 rank tiles
OT = O // P  # 16 output tiles
NC_CHUNK = 512  # moving free dim per matmul (PSUM bank, fp32 max)
TC = TOK // NC_CHUNK  # 2 token chunks
N_WARMUP = 20  # dummy matmuls (N=256) to trip the PE clock-gate

_CACHE = {}


def _install_ntff_shim():
    """Register antenv.axon_hooks so run_bass_kernel_spmd(trace=True) can
    profile under axon. Safe no-op if anything is unavailable."""
    try:
        if "antenv.axon_hooks" in sys.modules:
            return
        mod = types.ModuleType("antenv.axon_hooks")
        mod._hook = None
        mod.set_axon_ntff_profile_hook = lambda h: setattr(mod, "_hook", h)
        mod.get_axon_ntff_profile_hook = lambda: mod._hook
        sys.modules["antenv.axon_hooks"] = mod
        from trn_agent_boot.trn_boot import _ntff_profile_via_ctypes

        mod._hook = _ntff_profile_via_ctypes("/opt/axon/libaxon_pjrt.so")
    except Exception:
        pass


def _degree_plan(ranges):
    """Per-degree packing plan from the block-sparsity ranges.

    Returns (plans, nblk_total) where plans[i] = (groups, nblk, chunks):
      groups: list of (kt, lo, hi, goff) — goff is the block offset of the
              group inside the degree's packed weight tile
      nblk:   total packed blocks for this degree
      chunks: list of (block_start, block_count) DMA splits (group-aligned)
    """
    plans = []
    total = 0
    for i in range(DEGREE):
        groups = []
        off = 0
        for kt in range(KT):
            rng = ranges[i][kt]
            if rng is None:
                continue
            lo, hi = rng
            groups.append((kt, lo, hi, off))
            off += hi - lo + 1
        nblk = off
        # DMA splits: degrees 0-1 are consumed while z is still streaming
        # in — per-group chunks keep the matmul stream fed at fine grain.
        # Later degrees prefetch far ahead of use: two chunks suffice.
        bounds = [g[3] for g in groups] + [nblk]
        if i <= 1:
            cuts = bounds[:-1]
        else:
            half = min(
                (b for b in bounds[1:-1] if b > 0),
                key=lambda b: abs(b - nblk // 2),
                default=None,
            )
            cuts = [0] + ([half] if half else [])
        cuts = sorted(set(c for c in cuts if c < nblk))
        chunks = [
            (c, (cuts[j + 1] if j + 1 < len(cuts) else nblk) - c)
            for j, c in enumerate(cuts)
        ]
        plans.append((groups, nblk, chunks))
        total += nblk
    return plans, total


def _build(ranges):
    """ranges[i][kt] = (dt_lo, dt_hi) inclusive active range, or None if the
    whole (degree, rank-tile) block row is zero."""
    import concourse.tile as tile
    from concourse import bacc, mybir

    f32 = mybir.dt.float32
    bf16 = mybir.dt.bfloat16
    ADD = mybir.AluOpType.add
    MULT = mybir.AluOpType.mult
    COPY = mybir.ActivationFunctionType.Copy
    IDENT = mybir.ActivationFunctionType.Identity

    plans, nblk_total = _degree_plan(ranges)

    nc = bacc.Bacc("TRN2", target_bir_lowering=False, debug=False)

    # z.T per core, tile-major so each dt slice is a contiguous 256 KB
    # DMA: [dt, di, t] = z[t, dt*P + di]
    z_d = nc.dram_tensor("z", [DT, P, TOK], bf16, kind="ExternalInput")
    # Packed active weight blocks, all degrees concatenated:
    # [di, (blk)*P + ki]; blk enumerates (degree, kt, dt) in program order.
    w_d = nc.dram_tensor("w", [P, nblk_total * P], bf16, kind="ExternalInput")
    # C_w tiled: [ot, ki, kt*P + oi] = C_w[ot*P+oi, kt*P+ki]
    c_d = nc.dram_tensor("c", [OT, P, KT * P], bf16, kind="ExternalInput")
    # C_b tiled: [oi, ot] = C_b[ot*P + oi]
    cb_d = nc.dram_tensor("cb", [P, OT], f32, kind="ExternalInput")
    # x.T tile-major: [ot, oi, t] = x.T[ot*P + oi, t]
    x_d = nc.dram_tensor("x", [OT, P, TOK], f32, kind="ExternalOutput")

    z_ap, w_ap, c_ap, cb_ap, x_ap = (t.ap() for t in (z_d, w_d, c_d, cb_d, x_d))

    with tile.TileContext(nc) as tc, ExitStack() as ctx:
        zpool = ctx.enter_context(tc.tile_pool(name="z", bufs=1))
        accpool = ctx.enter_context(tc.tile_pool(name="acc", bufs=1))
        wpool = ctx.enter_context(tc.tile_pool(name="w", bufs=2))
        cpool = ctx.enter_context(tc.tile_pool(name="c", bufs=4))
        cbpool = ctx.enter_context(tc.tile_pool(name="cb", bufs=1))
        xpool = ctx.enter_context(tc.tile_pool(name="xt", bufs=2))
        wupool = ctx.enter_context(tc.tile_pool(name="wu", bufs=1))
        pspool = ctx.enter_context(tc.tile_pool(name="ps", bufs=4, space="PSUM"))

        # Single resident tiles; subtile dep tracking keeps readers of each
        # region independent.
        z_sb = zpool.tile([P, DT * TOK], bf16, tag="z")
        acc_sb = accpool.tile([P, KT * TOK], bf16, tag="acc")
        cb_sb = cbpool.tile([P, OT], f32)

        def z_r(dt_, tcx_=None):
            if tcx_ is None:
                return z_sb[:, dt_ * TOK : (dt_ + 1) * TOK]
            lo_ = dt_ * TOK + tcx_ * NC_CHUNK
            return z_sb[:, lo_ : lo_ + NC_CHUNK]

        def acc_r(kt_, tcx_=None):
            if tcx_ is None:
                return acc_sb[:, kt_ * TOK : (kt_ + 1) * TOK]
            lo_ = kt_ * TOK + tcx_ * NC_CHUNK
            return acc_sb[:, lo_ : lo_ + NC_CHUNK]

        # PE warmup: a burst of dummy matmuls on zeroed SBUF keeps the PE
        # busy while the first input DMAs land, tripping the HAM clock-gate
        # to full rate before real work starts.
        wu = wupool.tile([P, 256], bf16)
        nc.gpsimd.memset(wu[:], 0.0)
        wups = pspool.tile([P, TOK], f32, tag="ps")
        for _ in range(N_WARMUP):
            nc.tensor.matmul(wups[:, :256], wu[:, :P], wu[:], start=True, stop=True)

        # Issue z DMAs lazily, right before the first group that reads each
        # dt slice; the early (head-critical) slices split per token chunk.
        # z is spread round-robin across three otherwise-idle DMA queues —
        # one queue's ~130 GB/s cannot keep up with degree-0 consumption.
        z_issued = [False] * DT
        z_queues = [nc.gpsimd, nc.scalar]

        def ensure_z(lo_, hi_):
            for dt_ in range(lo_, hi_ + 1):
                if not z_issued[dt_]:
                    q = z_queues[dt_ % len(z_queues)]
                    nsp = 2 if dt_ < 8 else 1
                    for cx in range(nsp):
                        w_ = TOK // nsp
                        q.dma_start(
                            z_sb[:, dt_ * TOK + cx * w_ : dt_ * TOK + (cx + 1) * w_],
                            z_ap[dt_][:, cx * w_ : (cx + 1) * w_],
                        )
                    z_issued[dt_] = True

        # Degree chain over acc[kt-block, tokens].
        wbase = 0
        for i in range(DEGREE):
            groups, nblk, chunks = plans[i]
            if nblk == 0:
                for kt in range(KT):
                    if ranges[i][kt] is None and i == 0:
                        nc.gpsimd.memset(acc_r(kt), 0.0)
                continue
            w_sb = wpool.tile([P, nblk * P], bf16, tag="w", name=f"wdeg{i}")
            for cs, cn in chunks:
                nc.sync.dma_start(
                    w_sb[:, cs * P : (cs + cn) * P],
                    w_ap[:, (wbase + cs) * P : (wbase + cs + cn) * P],
                )
            if i == 0:
                for kt in range(KT):
                    if ranges[i][kt] is None:
                        nc.gpsimd.memset(acc_r(kt), 0.0)
            for kt, lo, hi, goff in groups:
                ndt = hi - lo + 1
                ensure_z(lo, hi)
                ps = pspool.tile([P, TOK], f32, tag="ps")
                # Steady state: dt outer / chunk inner — each weight tile is
                # loaded once and feeds both token chunks back-to-back.
                # Early degree-0 groups run chunk-outer instead: their first
                # half only needs the (faster-arriving) first token chunk of
                # each z slice, easing the startup DMA crunch.
                early = i == 0 and kt < 8
                if early:
                    order = [(j, tcx) for tcx in range(TC) for j in range(ndt)]
                else:
                    order = [(j, tcx) for j in range(ndt) for tcx in range(TC)]
                for j, tcx in order:
                    nc.tensor.matmul(
                        ps[:, tcx * NC_CHUNK : (tcx + 1) * NC_CHUNK],
                        w_sb[:, (goff + j) * P : (goff + j + 1) * P],
                        z_r(lo + j, tcx),
                        start=(j == 0),
                        stop=(j == ndt - 1),
                    )
                if i == 0:
                    # PSUM -> SBUF copy on the Activation engine
                    nc.scalar.activation(acc_r(kt), ps[:], COPY)
                else:
                    # acc = (mm + 1) * acc  — one DVE op
                    nc.vector.scalar_tensor_tensor(
                        acc_r(kt), ps[:], 1.0, acc_r(kt), ADD, MULT
                    )
            wbase += nblk

        # Final projection: x.T[ot-block] = C_w @ acc + C_b
        nc.sync.dma_start(cb_sb[:], cb_ap)
        for ot in range(OT):
            c_sb = cpool.tile([P, KT * P], bf16, tag="c")
            nc.sync.dma_start(c_sb[:], c_ap[ot])
            ps = pspool.tile([P, TOK], f32, tag="ps")
            for kt in range(KT):
                for tcx in range(TC):
                    nc.tensor.matmul(
                        ps[:, tcx * NC_CHUNK : (tcx + 1) * NC_CHUNK],
                        c_sb[:, kt * P : (kt + 1) * P],
                        acc_r(kt, tcx),
                        start=(kt == 0),
                        stop=(kt == KT - 1),
                    )
            xt = xpool.tile([P, TOK], f32, tag="xt")
            # Finer-grained drain near the end so the final bias-add + DMA
            # pipeline empties quickly after the last matmul.
            nsplit = 4 if ot == OT - 1 else (2 if ot == OT - 2 else 1)
            step = TOK // nsplit
            for h in range(nsplit):
                sl = slice(h * step, (h + 1) * step)
                nc.scalar.activation(
                    xt[:, sl], ps[:, sl], IDENT, bias=cb_sb[:, ot : ot + 1]
                )
                # DMAs on queues other than scalar so the descriptor pushes
                # don't serialize with the Activation ops at the drain.
                xq = nc.gpsimd if (ot + h) % 2 == 0 else nc.sync
                xq.dma_start(x_ap[ot][:, sl], xt[:, sl])

    nc.compile()
    return nc


def kernel(z, U, masks, C_w, C_b):
    import ml_dtypes
    from concourse.bass_utils import run_bass_kernel_spmd

    if os.environ.get("BASS_TRACE"):
        _install_ntff_shim()

    bf16 = ml_dtypes.bfloat16
    lead = z.shape[:-1]
    zf = np.ascontiguousarray(np.asarray(z, dtype=np.float32).reshape(-1, D))
    W = np.asarray(masks, dtype=np.float32) * np.asarray(U, dtype=np.float32)
    C_w = np.asarray(C_w, dtype=np.float32)
    C_b = np.asarray(C_b, dtype=np.float32)

    # Detect all-zero 128x128 blocks of W; build per-(degree, rank-tile)
    # contraction ranges. Only provably-zero blocks are skipped.
    blk = (
        np.abs(W.reshape(DEGREE, KT, P, DT, P)).max(axis=(2, 4)) > 0.0
    )  # [i, kt, dt]
    ranges = []
    for i in range(DEGREE):
        row = []
        for kt in range(KT):
            nz = np.flatnonzero(blk[i, kt])
            row.append((int(nz[0]), int(nz[-1])) if len(nz) else None)
        ranges.append(tuple(row))
    ranges = tuple(ranges)

    plans, nblk_total = _degree_plan(ranges)

    # Packed weight layout: for each degree, active groups' blocks
    # concatenated; block (i,kt,dt) -> [di, blk*P + ki].
    Wr = W.reshape(DEGREE, KT, P, DT, P)  # [i, kt, ki, dt, di]
    parts = []
    for i in range(DEGREE):
        for kt, lo, hi, _ in plans[i][0]:
            blkw = Wr[i, kt][:, lo : hi + 1, :]  # [ki, ndt, di]
            parts.append(blkw.transpose(2, 1, 0).reshape(P, -1))  # [di, ndt*P]
    w_dev = (
        np.concatenate(parts, axis=1).astype(bf16)
        if parts
        else np.zeros((P, 0), bf16)
    )
    w_dev = np.ascontiguousarray(w_dev)
    assert w_dev.shape[1] == nblk_total * P

    c_dev = np.ascontiguousarray(
        C_w.reshape(OT, P, KT, P).transpose(0, 3, 2, 1)
    ).reshape(OT, P, KT * P).astype(bf16)
    cb_dev = np.ascontiguousarray(C_b.reshape(OT, P).T)

    in_maps = []
    for c in range(N_CORES):
        zs = zf[c * TOK : (c + 1) * TOK]  # [TOK, D]
        z_dev = np.ascontiguousarray(zs.T.reshape(DT, P, TOK)).astype(bf16)
        in_maps.append({"z": z_dev, "w": w_dev, "c": c_dev, "cb": cb_dev})

    if _CACHE.get("ranges") != ranges:
        _CACHE["nc"] = _build(ranges)
        _CACHE["ranges"] = ranges
    nc = _CACHE["nc"]

    res = run_bass_kernel_spmd(nc, in_maps, core_ids=list(range(N_CORES)))
    _CACHE["last_result"] = res

    # per-core x is [OT, P, TOK]; rearrange to [TOK, O]
    parts = [
        res.results[c]["x"].reshape(O, TOK).T for c in range(N_CORES)
    ]
    x = np.concatenate(parts, axis=0)
    return x.reshape(*lead, O)
